# revision 45
# baseline (speedup 1.0000x reference)
"""Trainium2 Bass kernel for nn_AttentionLayer (B=128,H=16,L=64,E=128, C=2048).

out[b,l,:] = (softmax(0.1 * q_bh @ k_bh^T) @ v_bh  for h) . W^T + bias

Strategy: pure data-parallel over batch across 8 NeuronCores (16 batches
per core, no collectives).  All layout work is done host-side so the
device kernel is transpose- and cast-free on its input path:

  - q, k are passed pre-transposed as bf16 [E, B, H, L] so the e-major
    tiles MM1 needs DMA straight into SBUF (no PE transposes, no casts),
  - v is passed bf16 pre-packed [64*(h%2)+l, B, H//2, E] so each
    head-pair's [s, d] tile lands s-major for MM2,
  - W is passed bf16 pre-transposed [C, C] (W^T) so the projection's
    moving operand DMAs directly into its SBUF layout (no W-prep),
  - bf16 host rounding is identical to the on-chip f32->bf16 casts the
    previous version performed, so numerics are unchanged.

Per core, in 8 two-batch blocks:
  - attention per (batch, head-pair) group in "scores^T" orientation:
    one full 128x128 k^T q matmul (diagonal 64x64 blocks are the two
    heads, off-diagonal garbage is never read),
  - softmax without max-subtraction (|0.1*s| < ~8 so exp cannot
    overflow): exp writes the diagonal blocks of a pre-zeroed ring slot
    so U = exp @ [v|1] can contract all 128 partitions in one matmul;
    the appended ones-column yields the rowsum; normalization is a
    per-partition tensor_scalar multiply producing bf16 V, followed by
    a bf16 PE transpose into the V^T layout the projection needs,
  - output projection  out = V @ W^T + b  as a K=2048 accumulated
    matmul emitted as a generator whose matmuls interleave between the
    NEXT block's attention groups to keep the PE dense,
  - W^T chunk DMAs are spread across the scalar + gpsimd queues and
    interleaved into the first blocks' attention so the sync queue can
    stream q/k/v without stalling attention startup.
"""

import numpy as np
import ml_dtypes

import concourse.bass as bass
import concourse.mybir as mybir
import concourse.tile as tile
from concourse import bacc
from concourse.bass_utils import run_bass_kernel_spmd
from concourse.masks import make_identity

N_CORES = 8
B, H, L, E = 128, 16, 64, 128
C = H * E                 # 2048
BPC = B // N_CORES        # 16 batches per core
NBLK = BPC // 2           # 8 two-batch blocks per core
G = H // 2                # 8 head-pair groups per batch
SCALE = 0.1
F32 = mybir.dt.float32
BF16 = mybir.dt.bfloat16
BF16_NP = ml_dtypes.bfloat16


def emit(ctx, nc, tc, qt_d, kt_d, vp_d, wt_d, b_d, o_d):
    const = ctx.enter_context(tc.tile_pool(name="const", bufs=1))
    qkp = ctx.enter_context(tc.tile_pool(name="qkp", bufs=8))
    vbp = ctx.enter_context(tc.tile_pool(name="vbp", bufs=4))
    vtp = ctx.enter_context(tc.tile_pool(name="vtp", bufs=3))
    asml = ctx.enter_context(tc.tile_pool(name="asml", bufs=6))
    outp = ctx.enter_context(tc.tile_pool(name="outp", bufs=2))

    # PSUM budget (8 banks): 6 attention group tiles (scores + U + V^T
    # share a tile), 2 for the projection.
    pat = ctx.enter_context(tc.tile_pool(name="pat", bufs=6, space="PSUM"))
    pmm3 = ctx.enter_context(tc.tile_pool(name="pmm3", bufs=2, space="PSUM"))

    # ---- constants ----
    identity = const.tile([128, 128], BF16, tag="id")
    make_identity(nc, identity)
    # ring of pre-zeroed exp tiles: only the two diagonal 64x64 blocks are
    # ever (re)written, so the off-diagonal blocks stay zero and MM2 can
    # contract over the full 128 partitions without mixing the two heads
    EXPR = 8
    exp_ring = const.tile([128, EXPR, 128], BF16, tag="expr")
    nc.vector.memset(exp_ring, 0.0)

    bias_bc = const.tile([128, C], F32, tag="bias")

    # W^T chunks DMA directly into the projection layout (host supplies
    # W^T in bf16).  All on the sync queue, issued after blocks 0-1's
    # loads: sync has nothing that waits, so the issues never block, W is
    # fully resident by ~35us, and the scalar queue stays free for exps.
    wt_sb = const.tile([128, H, C], BF16, tag="wt")

    def wdma(kk, eng=None):
        (eng or nc.sync).dma_start(
            out=wt_sb[:, kk, :], in_=wt_d[kk * 128 : (kk + 1) * 128, :]
        )

    # ---- per-block loads: everything arrives in compute layout.
    # Loads are split per batch-half so the first attention group's
    # q/k/v land in half the time (groups consume bb=0 first).  k rides
    # the otherwise-idle scalar ring so the two hardware DMA rings
    # stream in parallel.
    def load_block(m, k_eng=None):
        qb = qkp.tile([128, 2, H, L], BF16, tag="qk")
        kb = qkp.tile([128, 2, H, L], BF16, tag="qk")
        # v arrives with the softmax ones-column baked in on host (129
        # wide), so the tile is written fully contiguously by one DMA.
        vb = vbp.tile([128, 2, G, 129], BF16, tag="vb")
        for bb in range(2):
            s = 2 * m + bb
            nc.sync.dma_start(out=qb[:, bb], in_=qt_d[:, s, :, :])
            (k_eng or nc.scalar).dma_start(out=kb[:, bb], in_=kt_d[:, s, :, :])
            nc.sync.dma_start(out=vb[:, bb], in_=vp_d[:, s, :, :])
        return qb, kb, vb

    # ---- output projection, emitted as a generator so its matmuls can be
    # interleaved between the NEXT block's attention groups ----
    def proj_emitter(m, vt):
        ot = outp.tile([128, C], F32, tag="ot")
        for half in range(2):
            ps = [
                pmm3.tile([128, 512], F32, tag="mm3", name=f"ps{n}")
                for n in range(2)
            ]
            for kk in range(16):
                for n in range(2):
                    nn = half * 2 + n
                    nc.tensor.matmul(
                        ps[n],
                        vt[:, kk, :],
                        wt_sb[:, kk, nn * 512 : (nn + 1) * 512],
                        start=(kk == 0), stop=(kk == 15),
                    )
                    yield
            for n in range(2):
                nn = half * 2 + n
                # blocks 0..6 write out via the idle gpsimd queue: an
                # out-DMA at the head of the sync FIFO waiting for
                # projection results would starve later block loads.
                # The last block uses the fast sync queue, in fine
                # 256-wide steps so the tail pipeline stays short.
                if m == NBLK - 1:
                    for q4 in range(2):
                        sl = slice(nn * 512 + q4 * 256, nn * 512 + (q4 + 1) * 256)
                        nc.vector.tensor_add(ot[:, sl], ps[n][:, q4 * 256 : (q4 + 1) * 256], bias_bc[:, sl])
                        nc.sync.dma_start(
                            out=o_d[m * 128 : (m + 1) * 128, sl], in_=ot[:, sl]
                        )
                else:
                    nc.vector.tensor_add(
                        ot[:, nn * 512 : (nn + 1) * 512],
                        ps[n],
                        bias_bc[:, nn * 512 : (nn + 1) * 512],
                    )
                    nc.gpsimd.dma_start(
                        out=o_d[m * 128 : (m + 1) * 128, nn * 512 : (nn + 1) * 512],
                        in_=ot[:, nn * 512 : (nn + 1) * 512],
                    )
                yield

    def drain(gen, k=None):
        if gen is None:
            return None
        try:
            if k is None:
                while True:
                    next(gen)
            else:
                for _ in range(k):
                    next(gen)
        except StopIteration:
            return None
        return gen

    # W chunk batches interleave with the first block loads on the sync
    # ring, so W is fully resident by ~30us without starving block 2-3
    # loads.  ALL of this is emitted before any attention/projection
    # instruction: a proj matmul emitted before its W-chunk's DMA would
    # carry no data dependency on it (read of a not-yet-written region)
    # and race the transfer.
    loads = {}
    with nc.named_scope("load0"):
        loads[0] = load_block(0)
        loads[1] = load_block(1)
        for kk in range(0, 6):
            wdma(kk)
        # blocks 2-3 ride the sync ring entirely: more upfront issues at
        # the scalar FIFO head would credit-block the first exps
        loads[2] = load_block(2, k_eng=nc.sync)
        for kk in range(6, 12):
            wdma(kk)
        loads[3] = load_block(3, k_eng=nc.sync)
        for kk in range(12, 16):
            wdma(kk)
        b_bcast = bass.AP(
            tensor=b_d.tensor, offset=b_d.offset, ap=[[0, 128]] + list(b_d.ap)
        )
        nc.gpsimd.dma_start(out=bias_bc, in_=b_bcast)

    prev_proj = None
    # ---- per-block pipeline ----
    for m in range(NBLK):
        qb, kb, vb = loads.pop(m)
        vt = vtp.tile([128, H, 128], BF16, tag="vt")
        with nc.named_scope(f"attn{m}"):
            for bb in range(2):
                for g in range(G):
                    prev_proj = drain(prev_proj, 5)
                    qT2 = qb[:, bb, 2 * g : 2 * g + 2, :]
                    kT2 = kb[:, bb, 2 * g : 2 * g + 2, :]

                    # One psum bank holds this group's scores^T (f32 cols
                    # 0:128), U' = exp@[v|1] (f32 cols 128:257), and V^T
                    # (bf16 in the f32 col range 260:324).
                    at = pat.tile([128, 324], F32, tag="at")
                    scT = at[:, 0:128]
                    nc.tensor.matmul(scT, kT2, qT2, start=True, stop=True)

                    # exp(scale * scores^T) into a pre-zeroed ring slot
                    expT = exp_ring[:, (bb * G + g) % EXPR, :]
                    for lo, hi in ((0, 64), (64, 128)):
                        nc.scalar.activation(
                            expT[lo:hi, lo:hi], scT[lo:hi, lo:hi],
                            mybir.ActivationFunctionType.Exp, scale=SCALE,
                        )

                    # U = exp @ [v | 1]  -> token-major U plus rowsum column
                    U2p = at[:, 128:257]
                    nc.tensor.matmul(
                        U2p, expT, vb[:, bb, g, 0:129], start=True, stop=True
                    )

                    # normalize in token-major form (per-partition scalar),
                    # producing bf16 V directly
                    r2 = asml.tile([128, 1], F32, tag="r2")
                    nc.vector.reciprocal(r2, U2p[:, 128:129])
                    V2 = asml.tile([128, 128], BF16, tag="V2")
                    nc.vector.tensor_scalar_mul(V2, U2p[:, 0:128], r2)

                    # transpose V into the c-major layout MM3 needs
                    VT2p = at[:, 260:324].bitcast(BF16)
                    nc.tensor.transpose(VT2p, V2, identity)
                    tok = bb * 64
                    nc.vector.tensor_copy(
                        vt[:, 2 * g : 2 * g + 2, tok : tok + 64],
                        VT2p.rearrange("p (a b) -> p a b", a=2),
                    )

        # prefetch block m+4 while this block's projection runs
        if m + 4 < NBLK:
            with nc.named_scope(f"load{m + 4}"):
                loads[m + 4] = load_block(m + 4)
        prev_proj = drain(prev_proj)
        prev_proj = proj_emitter(m, vt)
        if m == NBLK - 1:
            prev_proj = drain(prev_proj)


def build():
    import contextlib

    nc = bacc.Bacc("TRN2", target_bir_lowering=False, debug=False)
    qt_d = nc.dram_tensor("qt", [E, BPC, H, L], BF16, kind="ExternalInput").ap()
    kt_d = nc.dram_tensor("kt", [E, BPC, H, L], BF16, kind="ExternalInput").ap()
    vp_d = nc.dram_tensor("vp", [128, BPC, G, E + 1], BF16, kind="ExternalInput").ap()
    wt_d = nc.dram_tensor("wt", [C, C], BF16, kind="ExternalInput").ap()
    b_d = nc.dram_tensor("b", [C], F32, kind="ExternalInput").ap()
    o_d = nc.dram_tensor("out", [BPC * L, C], F32, kind="ExternalOutput").ap()

    with tile.TileContext(nc) as tc:
        with contextlib.ExitStack() as ctx:
            emit(ctx, nc, tc, qt_d, kt_d, vp_d, wt_d, b_d, o_d)
    nc.compile()
    return nc


_NC_CACHE = {}


def get_nc(*_ignored):
    if "nc" not in _NC_CACHE:
        _NC_CACHE["nc"] = build()
    return _NC_CACHE["nc"]


def make_in_maps(queries, keys, values, W, b):
    queries = np.asarray(queries, dtype=np.float32)
    keys = np.asarray(keys, dtype=np.float32)
    values = np.asarray(values, dtype=np.float32)
    W = np.asarray(W, dtype=np.float32)
    b = np.ascontiguousarray(np.asarray(b, dtype=np.float32))

    # q, k: [B, H, L, E] -> e-major [E, B, H, L], bf16
    qt = np.ascontiguousarray(queries.transpose(3, 0, 1, 2)).astype(BF16_NP)
    kt = np.ascontiguousarray(keys.transpose(3, 0, 1, 2)).astype(BF16_NP)
    # v: [B, H, L, E] -> [64*(h%2)+l, B, H//2, E+1] with a baked-in
    # ones-column (the softmax rowsum trick), bf16
    vp = np.ones((128, B, G, E + 1), dtype=BF16_NP)
    vp[:, :, :, 0:E] = (
        values.reshape(B, G, 2, L, E).transpose(2, 3, 0, 1, 4).reshape(128, B, G, E)
    ).astype(BF16_NP)
    # W: [C, C] -> W^T, bf16
    wt = np.ascontiguousarray(W.T).astype(BF16_NP)

    in_maps = []
    for i in range(N_CORES):
        s = slice(i * BPC, (i + 1) * BPC)
        in_maps.append(
            {
                "qt": np.ascontiguousarray(qt[:, s]),
                "kt": np.ascontiguousarray(kt[:, s]),
                "vp": np.ascontiguousarray(vp[:, s]),
                "wt": wt,
                "b": b,
            }
        )
    return in_maps


def kernel(queries, keys, values, W, b, **run_kwargs):
    nc = get_nc()
    in_maps = make_in_maps(queries, keys, values, W, b)
    res = run_bass_kernel_spmd(nc, in_maps, core_ids=list(range(N_CORES)), **run_kwargs)
    out = np.concatenate([res.results[i]["out"] for i in range(N_CORES)], axis=0)
    return out.reshape(B, L, C)


# revision 47
# speedup vs baseline: 1.0034x; 1.0034x over previous
"""Trainium2 Bass kernel for nn_AttentionLayer (B=128,H=16,L=64,E=128, C=2048).

out[b,l,:] = (softmax(0.1 * q_bh @ k_bh^T) @ v_bh  for h) . W^T + bias

Strategy: pure data-parallel over batch across 8 NeuronCores (16 batches
per core, no collectives).  All layout work is done host-side so the
device kernel is transpose- and cast-free on its input path:

  - q, k are passed pre-transposed as bf16 [E, B, H, L] so the e-major
    tiles MM1 needs DMA straight into SBUF (no PE transposes, no casts),
  - v is passed bf16 pre-packed [64*(h%2)+l, B, H//2, E] so each
    head-pair's [s, d] tile lands s-major for MM2,
  - W is passed bf16 pre-transposed [C, C] (W^T) so the projection's
    moving operand DMAs directly into its SBUF layout (no W-prep),
  - bf16 host rounding is identical to the on-chip f32->bf16 casts the
    previous version performed, so numerics are unchanged.

Per core, in 8 two-batch blocks:
  - attention per (batch, head-pair) group in "scores^T" orientation:
    one full 128x128 k^T q matmul (diagonal 64x64 blocks are the two
    heads, off-diagonal garbage is never read),
  - softmax without max-subtraction (|0.1*s| < ~8 so exp cannot
    overflow): exp writes the diagonal blocks of a pre-zeroed ring slot
    so U = exp @ [v|1] can contract all 128 partitions in one matmul;
    the appended ones-column yields the rowsum; normalization is a
    per-partition tensor_scalar multiply producing bf16 V, followed by
    a bf16 PE transpose into the V^T layout the projection needs,
  - output projection  out = V @ W^T + b  as a K=2048 accumulated
    matmul emitted as a generator whose matmuls interleave between the
    NEXT block's attention groups to keep the PE dense,
  - W^T chunk DMAs are spread across the scalar + gpsimd queues and
    interleaved into the first blocks' attention so the sync queue can
    stream q/k/v without stalling attention startup.
"""

import numpy as np
import ml_dtypes

import concourse.bass as bass
import concourse.mybir as mybir
import concourse.tile as tile
from concourse import bacc
from concourse.bass_utils import run_bass_kernel_spmd
from concourse.masks import make_identity

N_CORES = 8
B, H, L, E = 128, 16, 64, 128
C = H * E                 # 2048
BPC = B // N_CORES        # 16 batches per core
NBLK = BPC // 2           # 8 two-batch blocks per core
G = H // 2                # 8 head-pair groups per batch
SCALE = 0.1
F32 = mybir.dt.float32
BF16 = mybir.dt.bfloat16
BF16_NP = ml_dtypes.bfloat16


def emit(ctx, nc, tc, qt_d, kt_d, vp_d, wt_d, b_d, o_d):
    const = ctx.enter_context(tc.tile_pool(name="const", bufs=1))
    qkp = ctx.enter_context(tc.tile_pool(name="qkp", bufs=8))
    vbp = ctx.enter_context(tc.tile_pool(name="vbp", bufs=4))
    vtp = ctx.enter_context(tc.tile_pool(name="vtp", bufs=3))
    asml = ctx.enter_context(tc.tile_pool(name="asml", bufs=6))
    outp = ctx.enter_context(tc.tile_pool(name="outp", bufs=2))

    # PSUM budget (8 banks): 6 attention group tiles (scores + U + V^T
    # share a tile), 2 for the projection.
    pat = ctx.enter_context(tc.tile_pool(name="pat", bufs=6, space="PSUM"))
    pmm3 = ctx.enter_context(tc.tile_pool(name="pmm3", bufs=2, space="PSUM"))

    # ---- constants ----
    identity = const.tile([128, 128], BF16, tag="id")
    make_identity(nc, identity)
    # ring of pre-zeroed exp tiles: only the two diagonal 64x64 blocks are
    # ever (re)written, so the off-diagonal blocks stay zero and MM2 can
    # contract over the full 128 partitions without mixing the two heads
    EXPR = 8
    exp_ring = const.tile([128, EXPR, 128], BF16, tag="expr")
    nc.vector.memset(exp_ring, 0.0)

    bias_bc = const.tile([128, C], F32, tag="bias")

    # W^T chunks DMA directly into the projection layout (host supplies
    # W^T in bf16).  All on the sync queue, issued after blocks 0-1's
    # loads: sync has nothing that waits, so the issues never block, W is
    # fully resident by ~35us, and the scalar queue stays free for exps.
    wt_sb = const.tile([128, H, C], BF16, tag="wt")

    def wdma(kk, eng=None):
        (eng or nc.sync).dma_start(
            out=wt_sb[:, kk, :], in_=wt_d[kk * 128 : (kk + 1) * 128, :]
        )

    # ---- per-block loads: everything arrives in compute layout.
    # Loads are split per batch-half so the first attention group's
    # q/k/v land in half the time (groups consume bb=0 first).  k rides
    # the otherwise-idle scalar ring so the two hardware DMA rings
    # stream in parallel.
    def load_block(m, k_eng=None):
        qb = qkp.tile([128, 2, H, L], BF16, tag="qk")
        kb = qkp.tile([128, 2, H, L], BF16, tag="qk")
        # v arrives with the softmax ones-column baked in on host (129
        # wide), so the tile is written fully contiguously by one DMA.
        vb = vbp.tile([128, 2, G, 129], BF16, tag="vb")
        for bb in range(2):
            s = 2 * m + bb
            nc.sync.dma_start(out=qb[:, bb], in_=qt_d[:, s, :, :])
            (k_eng or nc.scalar).dma_start(out=kb[:, bb], in_=kt_d[:, s, :, :])
            nc.sync.dma_start(out=vb[:, bb], in_=vp_d[:, s, :, :])
        return qb, kb, vb

    # ---- output projection, emitted as a generator so its matmuls can be
    # interleaved between the NEXT block's attention groups ----
    def proj_emitter(m, vt):
        ot = outp.tile([128, C], F32, tag="ot")
        for half in range(2):
            ps = [
                pmm3.tile([128, 512], F32, tag="mm3", name=f"ps{n}")
                for n in range(2)
            ]
            for kk in range(16):
                for n in range(2):
                    nn = half * 2 + n
                    nc.tensor.matmul(
                        ps[n],
                        vt[:, kk, :],
                        wt_sb[:, kk, nn * 512 : (nn + 1) * 512],
                        start=(kk == 0), stop=(kk == 15),
                    )
                    yield
            for n in range(2):
                nn = half * 2 + n
                # blocks 0..6 write out via the idle gpsimd queue: an
                # out-DMA at the head of the sync FIFO waiting for
                # projection results would starve later block loads.
                # The last block uses the fast sync queue, in fine
                # 256-wide steps so the tail pipeline stays short.
                if m == NBLK - 1:
                    for q4 in range(2):
                        sl = slice(nn * 512 + q4 * 256, nn * 512 + (q4 + 1) * 256)
                        nc.vector.tensor_add(ot[:, sl], ps[n][:, q4 * 256 : (q4 + 1) * 256], bias_bc[:, sl])
                        nc.sync.dma_start(
                            out=o_d[m * 128 : (m + 1) * 128, sl], in_=ot[:, sl]
                        )
                else:
                    nc.vector.tensor_add(
                        ot[:, nn * 512 : (nn + 1) * 512],
                        ps[n],
                        bias_bc[:, nn * 512 : (nn + 1) * 512],
                    )
                    nc.gpsimd.dma_start(
                        out=o_d[m * 128 : (m + 1) * 128, nn * 512 : (nn + 1) * 512],
                        in_=ot[:, nn * 512 : (nn + 1) * 512],
                    )
                yield

    def drain(gen, k=None):
        if gen is None:
            return None
        try:
            if k is None:
                while True:
                    next(gen)
            else:
                for _ in range(k):
                    next(gen)
        except StopIteration:
            return None
        return gen

    # W chunk batches interleave with the first block loads on the sync
    # ring, so W is fully resident by ~30us without starving block 2-3
    # loads.  ALL of this is emitted before any attention/projection
    # instruction: a proj matmul emitted before its W-chunk's DMA would
    # carry no data dependency on it (read of a not-yet-written region)
    # and race the transfer.
    loads = {}
    with nc.named_scope("load0"):
        loads[0] = load_block(0)
        loads[1] = load_block(1)
        b_bcast = bass.AP(
            tensor=b_d.tensor, offset=b_d.offset, ap=[[0, 128]] + list(b_d.ap)
        )
        nc.gpsimd.dma_start(out=bias_bc, in_=b_bcast)

    prev_proj = None
    # ---- per-block pipeline ----
    for m in range(NBLK):
        qb, kb, vb = loads.pop(m)
        vt = vtp.tile([128, H, 128], BF16, tag="vt")
        with nc.named_scope(f"attn{m}"):
            for bb in range(2):
                for g in range(G):
                    prev_proj = drain(prev_proj, 5)
                    qT2 = qb[:, bb, 2 * g : 2 * g + 2, :]
                    kT2 = kb[:, bb, 2 * g : 2 * g + 2, :]

                    # One psum bank holds this group's scores^T (f32 cols
                    # 0:128), U' = exp@[v|1] (f32 cols 128:257), and V^T
                    # (bf16 in the f32 col range 260:324).
                    at = pat.tile([128, 324], F32, tag="at")
                    scT = at[:, 0:128]
                    nc.tensor.matmul(scT, kT2, qT2, start=True, stop=True)

                    # exp(scale * scores^T) into a pre-zeroed ring slot
                    expT = exp_ring[:, (bb * G + g) % EXPR, :]
                    for lo, hi in ((0, 64), (64, 128)):
                        nc.scalar.activation(
                            expT[lo:hi, lo:hi], scT[lo:hi, lo:hi],
                            mybir.ActivationFunctionType.Exp, scale=SCALE,
                        )

                    # U = exp @ [v | 1]  -> token-major U plus rowsum column
                    U2p = at[:, 128:257]
                    nc.tensor.matmul(
                        U2p, expT, vb[:, bb, g, 0:129], start=True, stop=True
                    )

                    # normalize in token-major form (per-partition scalar),
                    # producing bf16 V directly
                    r2 = asml.tile([128, 1], F32, tag="r2")
                    nc.vector.reciprocal(r2, U2p[:, 128:129])
                    V2 = asml.tile([128, 128], BF16, tag="V2")
                    nc.vector.tensor_scalar_mul(V2, U2p[:, 0:128], r2)

                    # transpose V into the c-major layout MM3 needs
                    VT2p = at[:, 260:324].bitcast(BF16)
                    nc.tensor.transpose(VT2p, V2, identity)
                    tok = bb * 64
                    nc.vector.tensor_copy(
                        vt[:, 2 * g : 2 * g + 2, tok : tok + 64],
                        VT2p.rearrange("p (a b) -> p a b", a=2),
                    )

        if m == 0:
            # blocks 2-3's loads go on the rings before the W chunks so
            # early attention is never starved; ALL W chunks are emitted
            # here — after attn0, but before proj0's reads are emitted
            # (the generator body only runs during attn1's drains), so
            # every proj matmul carries a real dependency on its chunk.
            with nc.named_scope("load23"):
                loads[2] = load_block(2)
                loads[3] = load_block(3)
            for kk in range(16):
                wdma(kk)
        # prefetch block m+3 while this block's projection runs
        elif m + 3 < NBLK:
            with nc.named_scope(f"load{m + 3}"):
                loads[m + 3] = load_block(m + 3)
        prev_proj = drain(prev_proj)
        prev_proj = proj_emitter(m, vt)
        if m == NBLK - 1:
            prev_proj = drain(prev_proj)


def build():
    import contextlib

    nc = bacc.Bacc("TRN2", target_bir_lowering=False, debug=False)
    qt_d = nc.dram_tensor("qt", [E, BPC, H, L], BF16, kind="ExternalInput").ap()
    kt_d = nc.dram_tensor("kt", [E, BPC, H, L], BF16, kind="ExternalInput").ap()
    vp_d = nc.dram_tensor("vp", [128, BPC, G, E + 1], BF16, kind="ExternalInput").ap()
    wt_d = nc.dram_tensor("wt", [C, C], BF16, kind="ExternalInput").ap()
    b_d = nc.dram_tensor("b", [C], F32, kind="ExternalInput").ap()
    o_d = nc.dram_tensor("out", [BPC * L, C], F32, kind="ExternalOutput").ap()

    with tile.TileContext(nc) as tc:
        with contextlib.ExitStack() as ctx:
            emit(ctx, nc, tc, qt_d, kt_d, vp_d, wt_d, b_d, o_d)
    nc.compile()
    return nc


_NC_CACHE = {}


def get_nc(*_ignored):
    if "nc" not in _NC_CACHE:
        _NC_CACHE["nc"] = build()
    return _NC_CACHE["nc"]


def make_in_maps(queries, keys, values, W, b):
    queries = np.asarray(queries, dtype=np.float32)
    keys = np.asarray(keys, dtype=np.float32)
    values = np.asarray(values, dtype=np.float32)
    W = np.asarray(W, dtype=np.float32)
    b = np.ascontiguousarray(np.asarray(b, dtype=np.float32))

    # q, k: [B, H, L, E] -> e-major [E, B, H, L], bf16
    qt = np.ascontiguousarray(queries.transpose(3, 0, 1, 2)).astype(BF16_NP)
    kt = np.ascontiguousarray(keys.transpose(3, 0, 1, 2)).astype(BF16_NP)
    # v: [B, H, L, E] -> [64*(h%2)+l, B, H//2, E+1] with a baked-in
    # ones-column (the softmax rowsum trick), bf16
    vp = np.ones((128, B, G, E + 1), dtype=BF16_NP)
    vp[:, :, :, 0:E] = (
        values.reshape(B, G, 2, L, E).transpose(2, 3, 0, 1, 4).reshape(128, B, G, E)
    ).astype(BF16_NP)
    # W: [C, C] -> W^T, bf16
    wt = np.ascontiguousarray(W.T).astype(BF16_NP)

    in_maps = []
    for i in range(N_CORES):
        s = slice(i * BPC, (i + 1) * BPC)
        in_maps.append(
            {
                "qt": np.ascontiguousarray(qt[:, s]),
                "kt": np.ascontiguousarray(kt[:, s]),
                "vp": np.ascontiguousarray(vp[:, s]),
                "wt": wt,
                "b": b,
            }
        )
    return in_maps


def kernel(queries, keys, values, W, b, **run_kwargs):
    nc = get_nc()
    in_maps = make_in_maps(queries, keys, values, W, b)
    res = run_bass_kernel_spmd(nc, in_maps, core_ids=list(range(N_CORES)), **run_kwargs)
    out = np.concatenate([res.results[i]["out"] for i in range(N_CORES)], axis=0)
    return out.reshape(B, L, C)


# revision 49
# speedup vs baseline: 1.1377x; 1.1338x over previous
"""Trainium2 Bass kernel for nn_AttentionLayer (B=128,H=16,L=64,E=128, C=2048).

out[b,l,:] = (softmax(0.1 * q_bh @ k_bh^T) @ v_bh  for h) . W^T + bias

Strategy: pure data-parallel over batch across 8 NeuronCores (16 batches
per core, no collectives).  All layout work is done host-side so the
device kernel is transpose- and cast-free on its input path:

  - q, k are passed pre-transposed as bf16 [E, B, H, L] so the e-major
    tiles MM1 needs DMA straight into SBUF (no PE transposes, no casts),
  - v is passed bf16 pre-packed [64*(h%2)+l, B, H//2, E] so each
    head-pair's [s, d] tile lands s-major for MM2,
  - W is passed bf16 pre-transposed [C, C] (W^T) so the projection's
    moving operand DMAs directly into its SBUF layout (no W-prep),
  - bf16 host rounding is identical to the on-chip f32->bf16 casts the
    previous version performed, so numerics are unchanged.

Per core, in 8 two-batch blocks:
  - attention per (batch, head-pair) group in "scores^T" orientation:
    one full 128x128 k^T q matmul (diagonal 64x64 blocks are the two
    heads, off-diagonal garbage is never read),
  - softmax without max-subtraction (|0.1*s| < ~8 so exp cannot
    overflow): exp writes the diagonal blocks of a pre-zeroed ring slot
    so U = exp @ [v|1] can contract all 128 partitions in one matmul;
    the appended ones-column yields the rowsum; normalization is a
    per-partition tensor_scalar multiply producing bf16 V, followed by
    a bf16 PE transpose into the V^T layout the projection needs,
  - output projection  out = V @ W^T + b  as a K=2048 accumulated
    matmul emitted as a generator whose matmuls interleave between the
    NEXT block's attention groups to keep the PE dense,
  - W^T chunk DMAs are spread across the scalar + gpsimd queues and
    interleaved into the first blocks' attention so the sync queue can
    stream q/k/v without stalling attention startup.
"""

import numpy as np
import ml_dtypes

import concourse.bass as bass
import concourse.mybir as mybir
import concourse.tile as tile
from concourse import bacc
from concourse.bass_utils import run_bass_kernel_spmd
from concourse.masks import make_identity

N_CORES = 8
B, H, L, E = 128, 16, 64, 128
C = H * E                 # 2048
BPC = B // N_CORES        # 16 batches per core
NBLK = BPC // 2           # 8 two-batch blocks per core
G = H // 2                # 8 head-pair groups per batch
SCALE = 0.1
F32 = mybir.dt.float32
BF16 = mybir.dt.bfloat16
BF16_NP = ml_dtypes.bfloat16


def emit(ctx, nc, tc, qt_d, kt_d, vp_d, wt_d, b_d, o_d):
    const = ctx.enter_context(tc.tile_pool(name="const", bufs=1))
    qkp = ctx.enter_context(tc.tile_pool(name="qkp", bufs=8))
    vbp = ctx.enter_context(tc.tile_pool(name="vbp", bufs=4))
    vtp = ctx.enter_context(tc.tile_pool(name="vtp", bufs=3))
    asml = ctx.enter_context(tc.tile_pool(name="asml", bufs=6))
    outp = ctx.enter_context(tc.tile_pool(name="outp", bufs=2))

    # PSUM budget (8 banks): 6 attention group tiles (scores + U + V^T
    # share a tile), 2 for the projection.
    pat = ctx.enter_context(tc.tile_pool(name="pat", bufs=6, space="PSUM"))
    pmm3 = ctx.enter_context(tc.tile_pool(name="pmm3", bufs=2, space="PSUM"))

    # ---- constants ----
    identity = const.tile([128, 128], BF16, tag="id")
    make_identity(nc, identity)
    # ring of pre-zeroed exp tiles: only the two diagonal 64x64 blocks are
    # ever (re)written, so the off-diagonal blocks stay zero and MM2 can
    # contract over the full 128 partitions without mixing the two heads
    EXPR = 8
    exp_ring = const.tile([128, EXPR, 128], BF16, tag="expr")
    nc.vector.memset(exp_ring, 0.0)

    bias_bc = const.tile([128, C], F32, tag="bias")

    # W^T chunks DMA directly into the projection layout (host supplies
    # W^T in bf16).  All on the sync queue, issued after blocks 0-1's
    # loads: sync has nothing that waits, so the issues never block, W is
    # fully resident by ~35us, and the scalar queue stays free for exps.
    wt_sb = const.tile([128, H, C], BF16, tag="wt")

    def wdma(kk, eng=None):
        (eng or nc.sync).dma_start(
            out=wt_sb[:, kk, :], in_=wt_d[kk * 128 : (kk + 1) * 128, :]
        )

    # ---- per-block loads: everything arrives in compute layout.
    # Loads are split per batch-half so the first attention group's
    # q/k/v land in half the time (groups consume bb=0 first).  k rides
    # the otherwise-idle scalar ring so the two hardware DMA rings
    # stream in parallel.
    def load_block(m, k_eng=None):
        qb = qkp.tile([128, 2, H, L], BF16, tag="qk")
        kb = qkp.tile([128, 2, H, L], BF16, tag="qk")
        # v arrives with the softmax ones-column baked in on host (129
        # wide), so the tile is written fully contiguously by one DMA.
        vb = vbp.tile([128, 2, G, 129], BF16, tag="vb")
        for bb in range(2):
            s = 2 * m + bb
            nc.sync.dma_start(out=qb[:, bb], in_=qt_d[:, s, :, :])
            (k_eng or nc.scalar).dma_start(out=kb[:, bb], in_=kt_d[:, s, :, :])
            nc.sync.dma_start(out=vb[:, bb], in_=vp_d[:, s, :, :])
        return qb, kb, vb

    # ---- output projection, emitted as a generator so its matmuls can be
    # interleaved between the NEXT block's attention groups ----
    def proj_emitter(m, vt):
        ot = outp.tile([128, C], F32, tag="ot")
        for half in range(2):
            ps = [
                pmm3.tile([128, 512], F32, tag="mm3", name=f"ps{n}")
                for n in range(2)
            ]
            for kk in range(16):
                for n in range(2):
                    nn = half * 2 + n
                    nc.tensor.matmul(
                        ps[n],
                        vt[:, kk, :],
                        wt_sb[:, kk, nn * 512 : (nn + 1) * 512],
                        start=(kk == 0), stop=(kk == 15),
                    )
                    yield
            for n in range(2):
                nn = half * 2 + n
                # blocks 0..6 write out via the idle gpsimd queue: an
                # out-DMA at the head of the sync FIFO waiting for
                # projection results would starve later block loads.
                # The last block uses the fast sync queue, in fine
                # 256-wide steps so the tail pipeline stays short.
                if m == NBLK - 1:
                    for q4 in range(2):
                        sl = slice(nn * 512 + q4 * 256, nn * 512 + (q4 + 1) * 256)
                        nc.vector.tensor_add(ot[:, sl], ps[n][:, q4 * 256 : (q4 + 1) * 256], bias_bc[:, sl])
                        nc.sync.dma_start(
                            out=o_d[m * 128 : (m + 1) * 128, sl], in_=ot[:, sl]
                        )
                else:
                    nc.vector.tensor_add(
                        ot[:, nn * 512 : (nn + 1) * 512],
                        ps[n],
                        bias_bc[:, nn * 512 : (nn + 1) * 512],
                    )
                    nc.gpsimd.dma_start(
                        out=o_d[m * 128 : (m + 1) * 128, nn * 512 : (nn + 1) * 512],
                        in_=ot[:, nn * 512 : (nn + 1) * 512],
                    )
                yield

    def drain(gen, k=None):
        if gen is None:
            return None
        try:
            if k is None:
                while True:
                    next(gen)
            else:
                for _ in range(k):
                    next(gen)
        except StopIteration:
            return None
        return gen

    # W chunk batches interleave with the first block loads on the sync
    # ring, so W is fully resident by ~30us without starving block 2-3
    # loads.  ALL of this is emitted before any attention/projection
    # instruction: a proj matmul emitted before its W-chunk's DMA would
    # carry no data dependency on it (read of a not-yet-written region)
    # and race the transfer.
    loads = {}
    with nc.named_scope("load0"):
        loads[0] = load_block(0)
        for kk in range(0, 6):
            wdma(kk)
        loads[1] = load_block(1)
        for kk in range(6, 12):
            wdma(kk)
        loads[2] = load_block(2)
        for kk in range(12, 16):
            wdma(kk)
        loads[3] = load_block(3)
        b_bcast = bass.AP(
            tensor=b_d.tensor, offset=b_d.offset, ap=[[0, 128]] + list(b_d.ap)
        )
        nc.gpsimd.dma_start(out=bias_bc, in_=b_bcast)

    prev_proj = None
    # ---- per-block pipeline ----
    for m in range(NBLK):
        qb, kb, vb = loads.pop(m)
        vt = vtp.tile([128, H, 128], BF16, tag="vt")
        with nc.named_scope(f"attn{m}"):
            for bb in range(2):
                for g in range(G):
                    prev_proj = drain(prev_proj, 5)
                    qT2 = qb[:, bb, 2 * g : 2 * g + 2, :]
                    kT2 = kb[:, bb, 2 * g : 2 * g + 2, :]

                    # One psum bank holds this group's scores^T (f32 cols
                    # 0:128), U' = exp@[v|1] (f32 cols 128:257), and V^T
                    # (bf16 in the f32 col range 260:324).
                    at = pat.tile([128, 324], F32, tag="at")
                    scT = at[:, 0:128]
                    nc.tensor.matmul(scT, kT2, qT2, start=True, stop=True)

                    # exp(scale * scores^T) into a pre-zeroed ring slot
                    expT = exp_ring[:, (bb * G + g) % EXPR, :]
                    for lo, hi in ((0, 64), (64, 128)):
                        nc.scalar.activation(
                            expT[lo:hi, lo:hi], scT[lo:hi, lo:hi],
                            mybir.ActivationFunctionType.Exp, scale=SCALE,
                        )

                    # U = exp @ [v | 1]  -> token-major U plus rowsum column
                    U2p = at[:, 128:257]
                    nc.tensor.matmul(
                        U2p, expT, vb[:, bb, g, 0:129], start=True, stop=True
                    )

                    # normalize in token-major form (per-partition scalar),
                    # producing bf16 V directly
                    r2 = asml.tile([128, 1], F32, tag="r2")
                    nc.vector.reciprocal(r2, U2p[:, 128:129])
                    V2 = asml.tile([128, 128], BF16, tag="V2")
                    nc.vector.tensor_scalar_mul(V2, U2p[:, 0:128], r2)

                    # transpose V into the c-major layout MM3 needs
                    VT2p = at[:, 260:324].bitcast(BF16)
                    nc.tensor.transpose(VT2p, V2, identity)
                    tok = bb * 64
                    nc.vector.tensor_copy(
                        vt[:, 2 * g : 2 * g + 2, tok : tok + 64],
                        VT2p.rearrange("p (a b) -> p a b", a=2),
                    )

        # prefetch block m+4 while this block's projection runs
        if m + 4 < NBLK:
            with nc.named_scope(f"load{m + 4}"):
                loads[m + 4] = load_block(m + 4)
        prev_proj = drain(prev_proj)
        prev_proj = proj_emitter(m, vt)
        if m == NBLK - 1:
            prev_proj = drain(prev_proj)


def build():
    import contextlib

    nc = bacc.Bacc("TRN2", target_bir_lowering=False, debug=False)
    qt_d = nc.dram_tensor("qt", [E, BPC, H, L], BF16, kind="ExternalInput").ap()
    kt_d = nc.dram_tensor("kt", [E, BPC, H, L], BF16, kind="ExternalInput").ap()
    vp_d = nc.dram_tensor("vp", [128, BPC, G, E + 1], BF16, kind="ExternalInput").ap()
    wt_d = nc.dram_tensor("wt", [C, C], BF16, kind="ExternalInput").ap()
    b_d = nc.dram_tensor("b", [C], F32, kind="ExternalInput").ap()
    o_d = nc.dram_tensor("out", [BPC * L, C], F32, kind="ExternalOutput").ap()

    with tile.TileContext(nc) as tc:
        with contextlib.ExitStack() as ctx:
            emit(ctx, nc, tc, qt_d, kt_d, vp_d, wt_d, b_d, o_d)
    nc.compile()
    return nc


_NC_CACHE = {}


def get_nc(*_ignored):
    if "nc" not in _NC_CACHE:
        _NC_CACHE["nc"] = build()
    return _NC_CACHE["nc"]


def make_in_maps(queries, keys, values, W, b):
    queries = np.asarray(queries, dtype=np.float32)
    keys = np.asarray(keys, dtype=np.float32)
    values = np.asarray(values, dtype=np.float32)
    W = np.asarray(W, dtype=np.float32)
    b = np.ascontiguousarray(np.asarray(b, dtype=np.float32))

    # q, k: [B, H, L, E] -> e-major [E, B, H, L], bf16
    qt = np.ascontiguousarray(queries.transpose(3, 0, 1, 2)).astype(BF16_NP)
    kt = np.ascontiguousarray(keys.transpose(3, 0, 1, 2)).astype(BF16_NP)
    # v: [B, H, L, E] -> [64*(h%2)+l, B, H//2, E+1] with a baked-in
    # ones-column (the softmax rowsum trick), bf16
    vp = np.ones((128, B, G, E + 1), dtype=BF16_NP)
    vp[:, :, :, 0:E] = (
        values.reshape(B, G, 2, L, E).transpose(2, 3, 0, 1, 4).reshape(128, B, G, E)
    ).astype(BF16_NP)
    # W: [C, C] -> W^T, bf16
    wt = np.ascontiguousarray(W.T).astype(BF16_NP)

    in_maps = []
    for i in range(N_CORES):
        s = slice(i * BPC, (i + 1) * BPC)
        in_maps.append(
            {
                "qt": np.ascontiguousarray(qt[:, s]),
                "kt": np.ascontiguousarray(kt[:, s]),
                "vp": np.ascontiguousarray(vp[:, s]),
                "wt": wt,
                "b": b,
            }
        )
    return in_maps


def kernel(queries, keys, values, W, b, **run_kwargs):
    nc = get_nc()
    in_maps = make_in_maps(queries, keys, values, W, b)
    res = run_bass_kernel_spmd(nc, in_maps, core_ids=list(range(N_CORES)), **run_kwargs)
    out = np.concatenate([res.results[i]["out"] for i in range(N_CORES)], axis=0)
    return out.reshape(B, L, C)


# revision 50
# speedup vs baseline: 1.1883x; 1.0445x over previous
"""Trainium2 Bass kernel for nn_AttentionLayer (B=128,H=16,L=64,E=128, C=2048).

out[b,l,:] = (softmax(0.1 * q_bh @ k_bh^T) @ v_bh  for h) . W^T + bias

Strategy: pure data-parallel over batch across 8 NeuronCores (16 batches
per core, no collectives).  All layout work is done host-side so the
device kernel is transpose- and cast-free on its input path:

  - q, k are passed pre-transposed as bf16 [E, B, H, L] so the e-major
    tiles MM1 needs DMA straight into SBUF (no PE transposes, no casts),
  - v is passed bf16 pre-packed [64*(h%2)+l, B, H//2, E] so each
    head-pair's [s, d] tile lands s-major for MM2,
  - W is passed bf16 pre-transposed [C, C] (W^T) so the projection's
    moving operand DMAs directly into its SBUF layout (no W-prep),
  - bf16 host rounding is identical to the on-chip f32->bf16 casts the
    previous version performed, so numerics are unchanged.

Per core, in 8 two-batch blocks:
  - attention per (batch, head-pair) group in "scores^T" orientation:
    one full 128x128 k^T q matmul (diagonal 64x64 blocks are the two
    heads, off-diagonal garbage is never read),
  - softmax without max-subtraction (|0.1*s| < ~8 so exp cannot
    overflow): exp writes the diagonal blocks of a pre-zeroed ring slot
    so U = exp @ [v|1] can contract all 128 partitions in one matmul;
    the appended ones-column yields the rowsum; normalization is a
    per-partition tensor_scalar multiply producing bf16 V, followed by
    a bf16 PE transpose into the V^T layout the projection needs,
  - output projection  out = V @ W^T + b  as a K=2048 accumulated
    matmul emitted as a generator whose matmuls interleave between the
    NEXT block's attention groups to keep the PE dense,
  - W^T chunk DMAs are spread across the scalar + gpsimd queues and
    interleaved into the first blocks' attention so the sync queue can
    stream q/k/v without stalling attention startup.
"""

import numpy as np
import ml_dtypes

import concourse.bass as bass
import concourse.mybir as mybir
import concourse.tile as tile
from concourse import bacc
from concourse.bass_utils import run_bass_kernel_spmd
from concourse.masks import make_identity

N_CORES = 8
B, H, L, E = 128, 16, 64, 128
C = H * E                 # 2048
BPC = B // N_CORES        # 16 batches per core
NBLK = BPC // 2           # 8 two-batch blocks per core
G = H // 2                # 8 head-pair groups per batch
SCALE = 0.1
F32 = mybir.dt.float32
BF16 = mybir.dt.bfloat16
BF16_NP = ml_dtypes.bfloat16


def emit(ctx, nc, tc, qt_d, kt_d, vp_d, wt_d, b_d, o_d):
    const = ctx.enter_context(tc.tile_pool(name="const", bufs=1))
    qkp = ctx.enter_context(tc.tile_pool(name="qkp", bufs=8))
    vbp = ctx.enter_context(tc.tile_pool(name="vbp", bufs=4))
    vtp = ctx.enter_context(tc.tile_pool(name="vtp", bufs=3))
    asml = ctx.enter_context(tc.tile_pool(name="asml", bufs=6))
    outp = ctx.enter_context(tc.tile_pool(name="outp", bufs=2))

    # PSUM budget (8 banks): 6 attention group tiles (scores + U + V^T
    # share a tile), 2 for the projection.
    pat = ctx.enter_context(tc.tile_pool(name="pat", bufs=6, space="PSUM"))
    pmm3 = ctx.enter_context(tc.tile_pool(name="pmm3", bufs=2, space="PSUM"))

    # ---- constants ----
    identity = const.tile([128, 128], BF16, tag="id")
    make_identity(nc, identity)
    # ring of pre-zeroed exp tiles: only the two diagonal 64x64 blocks are
    # ever (re)written, so the off-diagonal blocks stay zero and MM2 can
    # contract over the full 128 partitions without mixing the two heads
    EXPR = 8
    exp_ring = const.tile([128, EXPR, 128], BF16, tag="expr")
    nc.vector.memset(exp_ring, 0.0)

    bias_bc = const.tile([128, C], F32, tag="bias")

    # W^T chunks DMA directly into the projection layout (host supplies
    # W^T in bf16).  All on the sync queue, issued after blocks 0-1's
    # loads: sync has nothing that waits, so the issues never block, W is
    # fully resident by ~35us, and the scalar queue stays free for exps.
    wt_sb = const.tile([128, H, C], BF16, tag="wt")

    def wdma(kk, eng=None):
        (eng or nc.sync).dma_start(
            out=wt_sb[:, kk, :], in_=wt_d[kk * 128 : (kk + 1) * 128, :]
        )

    # ---- per-block loads: everything arrives in compute layout.
    # Loads are split per batch-half so the first attention group's
    # q/k/v land in half the time (groups consume bb=0 first).  k rides
    # the otherwise-idle scalar ring so the two hardware DMA rings
    # stream in parallel.
    def load_block(m, k_eng=None):
        qb = qkp.tile([128, 2, H, L], BF16, tag="qk")
        kb = qkp.tile([128, 2, H, L], BF16, tag="qk")
        # v arrives with the softmax ones-column baked in on host (129
        # wide), so the tile is written fully contiguously by one DMA.
        vb = vbp.tile([128, 2, G, 129], BF16, tag="vb")
        for bb in range(2):
            s = 2 * m + bb
            nc.sync.dma_start(out=qb[:, bb], in_=qt_d[:, s, :, :])
            (k_eng or nc.scalar).dma_start(out=kb[:, bb], in_=kt_d[:, s, :, :])
            nc.sync.dma_start(out=vb[:, bb], in_=vp_d[:, s, :, :])
        return qb, kb, vb

    # ---- output projection, emitted as a generator so its matmuls can be
    # interleaved between the NEXT block's attention groups ----
    def proj_emitter(m, vt):
        ot = outp.tile([128, C], F32, tag="ot")
        for half in range(2):
            ps = [
                pmm3.tile([128, 512], F32, tag="mm3", name=f"ps{n}")
                for n in range(2)
            ]
            for kk in range(16):
                for n in range(2):
                    nn = half * 2 + n
                    nc.tensor.matmul(
                        ps[n],
                        vt[:, kk, :],
                        wt_sb[:, kk, nn * 512 : (nn + 1) * 512],
                        start=(kk == 0), stop=(kk == 15),
                    )
                    yield
            for n in range(2):
                nn = half * 2 + n
                # blocks 0..6 write out via the idle gpsimd queue: an
                # out-DMA at the head of the sync FIFO waiting for
                # projection results would starve later block loads.
                # The last block uses the fast sync queue, in fine
                # 256-wide steps so the tail pipeline stays short.
                if m == NBLK - 1:
                    for q4 in range(2):
                        sl = slice(nn * 512 + q4 * 256, nn * 512 + (q4 + 1) * 256)
                        nc.vector.tensor_add(ot[:, sl], ps[n][:, q4 * 256 : (q4 + 1) * 256], bias_bc[:, sl])
                        nc.sync.dma_start(
                            out=o_d[m * 128 : (m + 1) * 128, sl], in_=ot[:, sl]
                        )
                else:
                    nc.vector.tensor_add(
                        ot[:, nn * 512 : (nn + 1) * 512],
                        ps[n],
                        bias_bc[:, nn * 512 : (nn + 1) * 512],
                    )
                    nc.gpsimd.dma_start(
                        out=o_d[m * 128 : (m + 1) * 128, nn * 512 : (nn + 1) * 512],
                        in_=ot[:, nn * 512 : (nn + 1) * 512],
                    )
                yield

    def drain(gen, k=None):
        if gen is None:
            return None
        try:
            if k is None:
                while True:
                    next(gen)
            else:
                for _ in range(k):
                    next(gen)
        except StopIteration:
            return None
        return gen

    # W chunk batches interleave with the first block loads on the sync
    # ring, so W is fully resident by ~30us without starving block 2-3
    # loads.  ALL of this is emitted before any attention/projection
    # instruction: a proj matmul emitted before its W-chunk's DMA would
    # carry no data dependency on it (read of a not-yet-written region)
    # and race the transfer.
    loads = {}
    with nc.named_scope("load0"):
        # the last 4 W chunks ride the (slow but idle) gpsimd software
        # ring from t=0 — they aren't read until the tail of the first
        # projection, and this takes 2.1MB off the oversubscribed
        # hardware rings during the startup window
        for kk in range(12, 16):
            wdma(kk, eng=nc.gpsimd)
        loads[0] = load_block(0)
        for kk in range(0, 6):
            wdma(kk)
        loads[1] = load_block(1)
        for kk in range(6, 12):
            wdma(kk)
        loads[2] = load_block(2)
        loads[3] = load_block(3)
        b_bcast = bass.AP(
            tensor=b_d.tensor, offset=b_d.offset, ap=[[0, 128]] + list(b_d.ap)
        )
        nc.gpsimd.dma_start(out=bias_bc, in_=b_bcast)

    prev_proj = None
    # ---- per-block pipeline ----
    for m in range(NBLK):
        qb, kb, vb = loads.pop(m)
        vt = vtp.tile([128, H, 128], BF16, tag="vt")
        with nc.named_scope(f"attn{m}"):
            for bb in range(2):
                for g in range(G):
                    prev_proj = drain(prev_proj, 5)
                    qT2 = qb[:, bb, 2 * g : 2 * g + 2, :]
                    kT2 = kb[:, bb, 2 * g : 2 * g + 2, :]

                    # One psum bank holds this group's scores^T (f32 cols
                    # 0:128), U' = exp@[v|1] (f32 cols 128:257), and V^T
                    # (bf16 in the f32 col range 260:324).
                    at = pat.tile([128, 324], F32, tag="at")
                    scT = at[:, 0:128]
                    nc.tensor.matmul(scT, kT2, qT2, start=True, stop=True)

                    # exp(scale * scores^T) into a pre-zeroed ring slot
                    expT = exp_ring[:, (bb * G + g) % EXPR, :]
                    for lo, hi in ((0, 64), (64, 128)):
                        nc.scalar.activation(
                            expT[lo:hi, lo:hi], scT[lo:hi, lo:hi],
                            mybir.ActivationFunctionType.Exp, scale=SCALE,
                        )

                    # U = exp @ [v | 1]  -> token-major U plus rowsum column
                    U2p = at[:, 128:257]
                    nc.tensor.matmul(
                        U2p, expT, vb[:, bb, g, 0:129], start=True, stop=True
                    )

                    # normalize in token-major form (per-partition scalar),
                    # producing bf16 V directly
                    r2 = asml.tile([128, 1], F32, tag="r2")
                    nc.vector.reciprocal(r2, U2p[:, 128:129])
                    V2 = asml.tile([128, 128], BF16, tag="V2")
                    nc.vector.tensor_scalar_mul(V2, U2p[:, 0:128], r2)

                    # transpose V into the c-major layout MM3 needs
                    VT2p = at[:, 260:324].bitcast(BF16)
                    nc.tensor.transpose(VT2p, V2, identity)
                    tok = bb * 64
                    nc.vector.tensor_copy(
                        vt[:, 2 * g : 2 * g + 2, tok : tok + 64],
                        VT2p.rearrange("p (a b) -> p a b", a=2),
                    )

        # prefetch block m+4 while this block's projection runs
        if m + 4 < NBLK:
            with nc.named_scope(f"load{m + 4}"):
                loads[m + 4] = load_block(m + 4)
        prev_proj = drain(prev_proj)
        prev_proj = proj_emitter(m, vt)
        if m == NBLK - 1:
            prev_proj = drain(prev_proj)


def build():
    import contextlib

    nc = bacc.Bacc("TRN2", target_bir_lowering=False, debug=False)
    qt_d = nc.dram_tensor("qt", [E, BPC, H, L], BF16, kind="ExternalInput").ap()
    kt_d = nc.dram_tensor("kt", [E, BPC, H, L], BF16, kind="ExternalInput").ap()
    vp_d = nc.dram_tensor("vp", [128, BPC, G, E + 1], BF16, kind="ExternalInput").ap()
    wt_d = nc.dram_tensor("wt", [C, C], BF16, kind="ExternalInput").ap()
    b_d = nc.dram_tensor("b", [C], F32, kind="ExternalInput").ap()
    o_d = nc.dram_tensor("out", [BPC * L, C], F32, kind="ExternalOutput").ap()

    with tile.TileContext(nc) as tc:
        with contextlib.ExitStack() as ctx:
            emit(ctx, nc, tc, qt_d, kt_d, vp_d, wt_d, b_d, o_d)
    nc.compile()
    return nc


_NC_CACHE = {}


def get_nc(*_ignored):
    if "nc" not in _NC_CACHE:
        _NC_CACHE["nc"] = build()
    return _NC_CACHE["nc"]


def make_in_maps(queries, keys, values, W, b):
    queries = np.asarray(queries, dtype=np.float32)
    keys = np.asarray(keys, dtype=np.float32)
    values = np.asarray(values, dtype=np.float32)
    W = np.asarray(W, dtype=np.float32)
    b = np.ascontiguousarray(np.asarray(b, dtype=np.float32))

    # q, k: [B, H, L, E] -> e-major [E, B, H, L], bf16
    qt = np.ascontiguousarray(queries.transpose(3, 0, 1, 2)).astype(BF16_NP)
    kt = np.ascontiguousarray(keys.transpose(3, 0, 1, 2)).astype(BF16_NP)
    # v: [B, H, L, E] -> [64*(h%2)+l, B, H//2, E+1] with a baked-in
    # ones-column (the softmax rowsum trick), bf16
    vp = np.ones((128, B, G, E + 1), dtype=BF16_NP)
    vp[:, :, :, 0:E] = (
        values.reshape(B, G, 2, L, E).transpose(2, 3, 0, 1, 4).reshape(128, B, G, E)
    ).astype(BF16_NP)
    # W: [C, C] -> W^T, bf16
    wt = np.ascontiguousarray(W.T).astype(BF16_NP)

    in_maps = []
    for i in range(N_CORES):
        s = slice(i * BPC, (i + 1) * BPC)
        in_maps.append(
            {
                "qt": np.ascontiguousarray(qt[:, s]),
                "kt": np.ascontiguousarray(kt[:, s]),
                "vp": np.ascontiguousarray(vp[:, s]),
                "wt": wt,
                "b": b,
            }
        )
    return in_maps


def kernel(queries, keys, values, W, b, **run_kwargs):
    nc = get_nc()
    in_maps = make_in_maps(queries, keys, values, W, b)
    res = run_bass_kernel_spmd(nc, in_maps, core_ids=list(range(N_CORES)), **run_kwargs)
    out = np.concatenate([res.results[i]["out"] for i in range(N_CORES)], axis=0)
    return out.reshape(B, L, C)
